# revision 36
# baseline (speedup 1.0000x reference)
"""Trainium2 Bass kernel for CompanySpecificHeads (MoE-style routed MLP heads).

Semantics (matching the reference):
    out[b] = gelu(z[b] @ W1[cid[b]] + b1[cid[b]]) @ W2[cid[b]] + b2[cid[b]]

Expert-parallel across 8 NeuronCores, 8 companies per core.

  * W1 streamed as float8 E3M4 (4 mantissa bits) with a power-of-2 prescale
    folded out in the gelu activation's scale: halves the dominant HBM
    traffic (8MB -> 4MB/core). End-to-end rel err ~1.3e-2 < 2e-2. Tokens
    stay fp16 (mixed-dtype matmul), psum accumulate fp32.
  * Exact per-slot token widths: companies sorted by token count into 8
    slots of 8 (one company per core per slot); slot width = max count in
    slot. All cores share one width vector (SPMD single program).
  * Weight delivery is split across TWO independent descriptor sources -
    early slots on the sync HWDGE ring, last slots via gpsimd SWDGE - since
    one queue caps at ~280GB/s while the fabric sustains ~425GB/s. The
    scalar(ACT) ring carries only tokens+consts, all issued before the
    gelu ACTIVATEs start to occupy that engine's queue.
  * b1 added by the vector engine (broadcast AP over the psum tile); gelu
    on ACT with scale=1/SCALE; the W2 dot stays on the PE, software-
    pipelined one company behind L1.
  * HAM clock: the PE boots throttled at 1.2GHz and un-throttles only after
    ~4-5.5us of sustained fp16-path matmul activity; e3m4 matmuls do NOT
    register (measured). A dense fp16 warmup stream covers boot->data-ready
    and flips the clock; 96-col fp16 keep-warm matmuls each slot hold it.
  * Output staged in SBUF, stored in two chunks on the (idle) sync ring so
    the bulk store's receipt latency overlaps the last slots' compute.
"""

import numpy as np

B, C, D, H = 4096, 64, 512, 1024
NCORES = 8
CPC = C // NCORES
KC = D // 128      # contraction chunks of 128
HC = H // 128      # h chunks of 128
SCALE = 16.0       # W1 prescale before e3m4 quantization
WARMUP = 12
WARMW = 512

_COMPILED = {}


def _build(widths):
    """Build the Bass/Tile program for per-slot token widths `widths`."""
    import concourse.bass as bass
    import concourse.bacc as bacc
    import concourse.mybir as mybir
    from concourse.tile import TileContext
    from contextlib import ExitStack

    f32 = mybir.dt.float32
    f16 = mybir.dt.float16
    f8e3 = mybir.dt.float8e3

    NSLOT = len(widths)
    Wmax = max(widths)
    B1VW = NSLOT * 2 * KC
    cum = np.concatenate([[0], np.cumsum(widths)])
    NTOT = int(cum[-1])

    gelu = mybir.ActivationFunctionType.Gelu

    nc = bacc.Bacc(None, target_bir_lowering=False)

    NPAIR = (NSLOT + 1) // 2
    zt_d = nc.dram_tensor("zt", [128, KC * NTOT], f16, kind="ExternalInput")
    # weights packed two companies per DMA: larger transfers run the queue
    # at ~341-400GB/s vs ~295 (0.5MB) / ~224 (0.26MB)
    w1_d = nc.dram_tensor("w1", [NPAIR, 128, 4 * KC * (H // 2)], f8e3,
                          kind="ExternalInput")
    # b1v[m, (s, g, k)] = SCALE*b1[gc_s, 512g+128k+m] (DVE-broadcast layout)
    b1_d = nc.dram_tensor("b1v", [128, B1VW], f16, kind="ExternalInput")
    w2_d = nc.dram_tensor("w2", [128, NSLOT * HC], f16, kind="ExternalInput")
    out_d = nc.dram_tensor("out", [1, NTOT], f32, kind="ExternalOutput")

    with TileContext(nc) as tc, ExitStack() as ctx:
        const = ctx.enter_context(tc.tile_pool(name="const", bufs=1))

        # Warmup scratch: memset on the (idle, fast) vector engine so the PE
        # can start ramping the HAM clock right after engine boot.
        wsc = const.tile([128, WARMW], f16)
        nc.vector.memset(wsc[:], 0.0)

        zall = const.tile([128, KC * NTOT], f16)
        b1v = const.tile([128, B1VW], f16)
        w2t = const.tile([128, NSLOT * HC], f16)
        w1p = ctx.enter_context(tc.tile_pool(name="w1p", bufs=1))
        w1prs = [w1p.tile([128, 2, 2, KC, H // 2], f8e3, name=f"w1_{p}")
                 for p in range(NPAIR)]
        # view for slot s: w1ts[s][:, g, k, hh]
        w1ts = [w1prs[s // 2][:, s % 2] for s in range(NSLOT)]

        # All weights on the sync ring in consumption order: queue fairness
        # is strict round-robin, so spreading weights across queues pulls
        # late-needed slots early and starves the early-critical ones.
        for p in range(NPAIR):
            nc.sync.dma_start(out=w1prs[p][:], in_=w1_d[p])

        # Tokens + small consts on the scalar ring, consumption order,
        # all issued before the first ACTIVATE occupies the ACT queue.
        zcuts = sorted({min(2, NSLOT), min(4, NSLOT), NSLOT})
        lo = 0
        zsegs = []
        for hi in zcuts:
            if hi > lo:
                zsegs.append((int(KC * cum[lo]), int(KC * cum[hi])))
                lo = hi
        a, b = zsegs[0]
        nc.scalar.dma_start(out=zall[:, a:b], in_=zt_d[:, a:b])
        nc.scalar.dma_start(out=b1v[:], in_=b1_d[:])
        if len(zsegs) > 1:
            a, b = zsegs[1]
            nc.scalar.dma_start(out=zall[:, a:b], in_=zt_d[:, a:b])
        nc.scalar.dma_start(out=w2t[:], in_=w2_d[:])
        if len(zsegs) > 2:
            a, b = zsegs[2]
            nc.scalar.dma_start(out=zall[:, a:b], in_=zt_d[:, a:b])

        # Staged per-slot outputs.
        oall = const.tile([1, NTOT], f32)

        hp = ctx.enter_context(tc.tile_pool(name="hp", bufs=6))
        pp = ctx.enter_context(tc.tile_pool(name="pp", bufs=5, space="PSUM"))
        opp = ctx.enter_context(tc.tile_pool(name="opp", bufs=2, space="PSUM"))
        wps = ctx.enter_context(tc.tile_pool(name="wps", bufs=1, space="PSUM"))

        wp = wps.tile([128, WARMW], f32)
        for _ in range(WARMUP):
            nc.tensor.matmul(wp[:], wsc[:, :128], wsc[:], start=True, stop=True)

        b1b = b1v[:].rearrange("p (s g k one) -> p s g k one",
                               s=NSLOT, g=2, one=1)

        def do_l2(s, W, off, hts):
            osum = opp.tile([1, Wmax], f32)
            for g in range(2):
                for j in range(KC):
                    jj = KC * g + j
                    nc.tensor.matmul(
                        osum[:, :W],
                        w2t[:, HC * s + jj:HC * s + jj + 1],
                        hts[g][:, j * W:(j + 1) * W],
                        start=(jj == 0),
                        stop=(jj == HC - 1),
                    )
            nc.vector.tensor_copy(oall[:, off:off + W], osum[:, :W])

        stored = [0]
        prev = None
        for s in range(NSLOT):
            W = widths[s]
            off = int(cum[s])
            zc = zall[:, KC * off:KC * (off + W)].rearrange(
                "p (k t) -> p k t", k=KC)
            w1t = w1ts[s]
            # fp16 keep-warm pulse so HAM never sees an idle window
            nc.tensor.matmul(wp[:, :96], wsc[:, :128], wsc[:, :96],
                             start=True, stop=True)
            hts = []
            for g in range(2):
                ps = pp.tile([128, KC * Wmax], f32)
                # exactly ONE start=True per psum bank: the start flag clears
                # has_written bits for the whole bank
                for k in range(KC):
                    for j in range(KC):
                        nc.tensor.matmul(
                            ps[:, j * W:(j + 1) * W],
                            w1t[:, g, k, 128 * j:128 * (j + 1)],
                            zc[:, k, :],
                            start=(k == 0 and j == 0),
                            stop=(k == KC - 1),
                        )
                psb = ps[:, 0:KC * W].rearrange("p (j t) -> p j t", j=KC)
                b1bc = b1b[:, s, g, :, :].to_broadcast((128, KC, W))
                nc.vector.tensor_add(psb, psb, b1bc)
                ht = hp.tile([128, KC * Wmax], f16)
                nc.scalar.activation(ht[:, 0:KC * W], ps[:, 0:KC * W], gelu,
                                     scale=1.0 / SCALE)
                hts.append(ht)
            if prev is not None:
                do_l2(*prev)
                # early bulk store: its issue + HBM receipt latency overlaps
                # the last slots' compute
                if prev[0] == NSLOT - 3 and NSLOT > 2:
                    so = int(cum[NSLOT - 2])
                    nc.scalar.dma_start(out=out_d[:, :so], in_=oall[:, :so])
                    stored[0] = so
            prev = (s, W, off, hts)
        do_l2(*prev)

        so = stored[0]
        nc.scalar.dma_start(out=out_d[:, so:], in_=oall[:, so:])

    nc.finalize()
    return nc


def _get_compiled(widths):
    key = tuple(widths)
    if key not in _COMPILED:
        _COMPILED[key] = _build(list(widths))
    return _COMPILED[key]


def kernel(z, company_id, W1, b1, W2, b2):
    import ml_dtypes
    from concourse.bass_utils import run_bass_kernel_spmd

    z = np.asarray(z, dtype=np.float32)
    cid = np.asarray(company_id).astype(np.int64).ravel()
    W1 = np.asarray(W1, dtype=np.float32)
    b1 = np.asarray(b1, dtype=np.float32)
    W2 = np.asarray(W2, dtype=np.float32)
    b2 = np.asarray(b2, dtype=np.float32)
    O = W2.shape[2]

    idx_by_company = [np.nonzero(cid == gc)[0] for gc in range(C)]

    # Segment any company with >128 tokens (rare) into <=128-token chunks.
    segs = []  # (gc, tok_start, seg_len)
    for gc in range(C):
        n = len(idx_by_company[gc])
        st = 0
        while st < n or (st == 0 and n == 0):
            ln = min(128, n - st)
            segs.append((gc, st, ln))
            st += max(ln, 1)
            if n == 0:
                break
    while len(segs) % NCORES != 0:
        segs.append((0, 0, 0))

    # Sort descending; slot k gets segs[8k:8k+8] (one per core); shared width.
    segs.sort(key=lambda t: -t[2])
    NSLOT = len(segs) // NCORES
    widths = []
    for k in range(NSLOT):
        mx = max(t[2] for t in segs[k * NCORES:(k + 1) * NCORES])
        widths.append(max(2, ((mx + 1) // 2) * 2))
    cum = np.concatenate([[0], np.cumsum(widths)])
    NTOT = int(cum[-1])
    B1VW = NSLOT * 2 * KC

    nc = _get_compiled(widths)

    in_maps = []
    core_slots = []
    for core in range(NCORES):
        slots = [segs[k * NCORES + core] for k in range(NSLOT)]
        core_slots.append(slots)

        NPAIR = (NSLOT + 1) // 2
        HALF = 2 * KC * (H // 2)
        zt = np.zeros((128, KC * NTOT), dtype=np.float16)
        w1 = np.zeros((NPAIR, 128, 2 * HALF), dtype=ml_dtypes.float8_e3m4)
        b1v = np.zeros((128, NSLOT, 2, KC), dtype=np.float16)
        w2h = np.zeros((128, NSLOT * HC), dtype=np.float16)

        for s, (gc, st, ln) in enumerate(slots):
            W = widths[s]
            if ln > 0:
                ix = idx_by_company[gc][st:st + ln]
                zb = np.zeros((KC, 128, W), dtype=np.float16)
                zb[:, :, :ln] = (
                    z[ix].reshape(ln, KC, 128).transpose(1, 2, 0)
                )
                zt[:, KC * cum[s]:KC * (cum[s] + W)] = (
                    zb.transpose(1, 0, 2).reshape(128, KC * W)
                )
            w1[s // 2][:, (s % 2) * HALF:(s % 2 + 1) * HALF] = (
                (W1[gc] * SCALE)
                .reshape(KC, 128, 2, H // 2)
                .transpose(1, 2, 0, 3)
                .reshape(128, HALF)
                .astype(ml_dtypes.float8_e3m4)
            )
            # b1v[m, s, g, k] = SCALE*b1[gc, 512g+128k+m]
            b1v[:, s] = (
                (b1[gc] * SCALE).reshape(2, KC, 128).transpose(2, 0, 1)
            ).astype(np.float16)
            w2h[:, HC * s:HC * (s + 1)] = (
                W2[gc, :, 0].reshape(HC, 128).T.astype(np.float16)
            )

        in_maps.append({
            "zt": np.ascontiguousarray(zt),
            "w1": np.ascontiguousarray(w1),
            "b1v": np.ascontiguousarray(b1v.reshape(128, B1VW)),
            "w2": np.ascontiguousarray(w2h),
        })

    res = run_bass_kernel_spmd(nc, in_maps, list(range(NCORES)))

    out = np.zeros((B, O), dtype=np.float32)
    for core in range(NCORES):
        core_out = res.results[core]["out"].reshape(-1)
        for s, (gc, st, ln) in enumerate(core_slots[core]):
            if ln == 0:
                continue
            ix = idx_by_company[gc][st:st + ln]
            out[ix, 0] = core_out[cum[s]:cum[s] + ln] + b2[gc, 0]
    return out


# revision 40
# speedup vs baseline: 1.3481x; 1.3481x over previous
"""Trainium2 Bass kernel for CompanySpecificHeads (MoE-style routed MLP heads).

Semantics (matching the reference):
    out[b] = gelu(z[b] @ W1[cid[b]] + b1[cid[b]]) @ W2[cid[b]] + b2[cid[b]]

Expert-parallel across 8 NeuronCores, 8 companies per core.

  * W1 streamed as float8 E3M4 (4 mantissa bits) with a power-of-2 prescale
    folded out in the gelu activation's scale: halves the dominant HBM
    traffic (8MB -> 4MB/core). End-to-end rel err ~1.3e-2 < 2e-2. Tokens
    stay fp16 (mixed-dtype matmul), psum accumulate fp32.
  * Exact per-slot token widths: companies sorted by token count into 8
    slots of 8 (one company per core per slot); slot width = max count in
    slot. All cores share one width vector (SPMD single program).
  * Weight delivery is split across TWO independent descriptor sources -
    early slots on the sync HWDGE ring, last slots via gpsimd SWDGE - since
    one queue caps at ~280GB/s while the fabric sustains ~425GB/s. The
    scalar(ACT) ring carries only tokens+consts, all issued before the
    gelu ACTIVATEs start to occupy that engine's queue.
  * b1 added by the vector engine (broadcast AP over the psum tile); gelu
    on ACT with scale=1/SCALE; the W2 dot stays on the PE, software-
    pipelined one company behind L1.
  * HAM clock: the PE boots throttled at 1.2GHz and un-throttles only after
    ~4-5.5us of sustained fp16-path matmul activity; e3m4 matmuls do NOT
    register (measured). A dense fp16 warmup stream covers boot->data-ready
    and flips the clock; 96-col fp16 keep-warm matmuls each slot hold it.
  * Output staged in SBUF, stored in two chunks on the (idle) sync ring so
    the bulk store's receipt latency overlaps the last slots' compute.
"""

import numpy as np

B, C, D, H = 4096, 64, 512, 1024
NCORES = 8
CPC = C // NCORES
KC = D // 128      # contraction chunks of 128
HC = H // 128      # h chunks of 128
SCALE = 16.0       # W1 prescale before e3m4 quantization
WARMUP = 12
WARMW = 512

_COMPILED = {}


def _build(widths):
    """Build the Bass/Tile program for per-slot token widths `widths`."""
    import concourse.bass as bass
    import concourse.bacc as bacc
    import concourse.mybir as mybir
    from concourse.tile import TileContext
    from contextlib import ExitStack

    f32 = mybir.dt.float32
    f16 = mybir.dt.float16
    f8e3 = mybir.dt.float8e3

    NSLOT = len(widths)
    Wmax = max(widths)
    B1VW = NSLOT * 2 * KC
    cum = np.concatenate([[0], np.cumsum(widths)])
    NTOT = int(cum[-1])

    gelu = mybir.ActivationFunctionType.Gelu

    nc = bacc.Bacc(None, target_bir_lowering=False)

    zt_d = nc.dram_tensor("zt", [128, KC * NTOT], f16, kind="ExternalInput")
    w1_d = nc.dram_tensor("w1", [NSLOT, 128, 2 * KC * (H // 2)], f8e3,
                          kind="ExternalInput")
    # b1v[m, (s, g, k)] = SCALE*b1[gc_s, 512g+128k+m] (DVE-broadcast layout)
    b1_d = nc.dram_tensor("b1v", [128, B1VW], f16, kind="ExternalInput")
    w2_d = nc.dram_tensor("w2", [128, NSLOT * HC], f16, kind="ExternalInput")
    out_d = nc.dram_tensor("out", [1, NTOT], f32, kind="ExternalOutput")

    with TileContext(nc) as tc, ExitStack() as ctx:
        const = ctx.enter_context(tc.tile_pool(name="const", bufs=1))

        # Warmup scratch: memset on the (idle, fast) vector engine so the PE
        # can start ramping the HAM clock right after engine boot.
        wsc = const.tile([128, WARMW], f16)
        nc.vector.memset(wsc[:], 0.0)

        zall = const.tile([128, KC * NTOT], f16)
        b1v = const.tile([128, B1VW], f16)
        w2t = const.tile([128, NSLOT * HC], f16)
        w1p = ctx.enter_context(tc.tile_pool(name="w1p", bufs=1))
        w1ts = [w1p.tile([128, 2, KC, H // 2], f8e3, name=f"w1_{s}")
                for s in range(NSLOT)]

        # All weights on the sync ring in consumption order: queue fairness
        # is strict round-robin, so spreading weights across queues pulls
        # late-needed slots early and starves the early-critical ones.
        for s in range(NSLOT):
            nc.sync.dma_start(out=w1ts[s][:], in_=w1_d[s])

        # Tokens + small consts on the scalar ring, consumption order,
        # all issued before the first ACTIVATE occupies the ACT queue.
        zcuts = sorted({min(2, NSLOT), min(4, NSLOT), NSLOT})
        lo = 0
        zsegs = []
        for hi in zcuts:
            if hi > lo:
                zsegs.append((int(KC * cum[lo]), int(KC * cum[hi])))
                lo = hi
        a, b = zsegs[0]
        nc.scalar.dma_start(out=zall[:, a:b], in_=zt_d[:, a:b])
        nc.scalar.dma_start(out=b1v[:], in_=b1_d[:])
        if len(zsegs) > 1:
            a, b = zsegs[1]
            nc.scalar.dma_start(out=zall[:, a:b], in_=zt_d[:, a:b])
        nc.scalar.dma_start(out=w2t[:], in_=w2_d[:])
        # the tail z segment (needed only from slot 4, ~17.6us) is issued
        # later, inside the slot loop, so its bytes don't steal SDMA-engine
        # share from the weight stream during the critical early window

        # Staged per-slot outputs.
        oall = const.tile([1, NTOT], f32)

        hp = ctx.enter_context(tc.tile_pool(name="hp", bufs=6))
        pp = ctx.enter_context(tc.tile_pool(name="pp", bufs=5, space="PSUM"))
        opp = ctx.enter_context(tc.tile_pool(name="opp", bufs=2, space="PSUM"))
        wps = ctx.enter_context(tc.tile_pool(name="wps", bufs=1, space="PSUM"))

        wp = wps.tile([128, WARMW], f32)
        for _ in range(WARMUP):
            nc.tensor.matmul(wp[:], wsc[:, :128], wsc[:], start=True, stop=True)

        b1b = b1v[:].rearrange("p (s g k one) -> p s g k one",
                               s=NSLOT, g=2, one=1)

        def do_l2(s, W, off, hts):
            osum = opp.tile([1, Wmax], f32)
            for g in range(2):
                for j in range(KC):
                    jj = KC * g + j
                    nc.tensor.matmul(
                        osum[:, :W],
                        w2t[:, HC * s + jj:HC * s + jj + 1],
                        hts[g][:, j * W:(j + 1) * W],
                        start=(jj == 0),
                        stop=(jj == HC - 1),
                    )
            nc.vector.tensor_copy(oall[:, off:off + W], osum[:, :W])

        stored = [0]
        prev = None
        for s in range(NSLOT):
            W = widths[s]
            off = int(cum[s])
            zc = zall[:, KC * off:KC * (off + W)].rearrange(
                "p (k t) -> p k t", k=KC)
            w1t = w1ts[s]
            # fp16 keep-warm pulse so HAM never sees an idle window
            nc.tensor.matmul(wp[:, :96], wsc[:, :128], wsc[:, :96],
                             start=True, stop=True)
            hts = []
            for g in range(2):
                ps = pp.tile([128, KC * Wmax], f32)
                # exactly ONE start=True per psum bank: the start flag clears
                # has_written bits for the whole bank
                for k in range(KC):
                    for j in range(KC):
                        nc.tensor.matmul(
                            ps[:, j * W:(j + 1) * W],
                            w1t[:, g, k, 128 * j:128 * (j + 1)],
                            zc[:, k, :],
                            start=(k == 0 and j == 0),
                            stop=(k == KC - 1),
                        )
                psb = ps[:, 0:KC * W].rearrange("p (j t) -> p j t", j=KC)
                b1bc = b1b[:, s, g, :, :].to_broadcast((128, KC, W))
                nc.vector.tensor_add(psb, psb, b1bc)
                ht = hp.tile([128, KC * Wmax], f16)
                nc.scalar.activation(ht[:, 0:KC * W], ps[:, 0:KC * W], gelu,
                                     scale=1.0 / SCALE)
                hts.append(ht)
            if prev is not None:
                do_l2(*prev)
                # early bulk store: its issue + HBM receipt latency overlaps
                # the last slots' compute
                if prev[0] == NSLOT - 3 and NSLOT > 2:
                    so = int(cum[NSLOT - 2])
                    nc.scalar.dma_start(out=out_d[:, :so], in_=oall[:, :so])
                    stored[0] = so
            prev = (s, W, off, hts)
            if s == 1 and len(zsegs) > 2:
                a, b = zsegs[2]
                nc.scalar.dma_start(out=zall[:, a:b], in_=zt_d[:, a:b])
        do_l2(*prev)

        so = stored[0]
        nc.scalar.dma_start(out=out_d[:, so:], in_=oall[:, so:])

    nc.finalize()
    return nc


def _get_compiled(widths):
    key = tuple(widths)
    if key not in _COMPILED:
        _COMPILED[key] = _build(list(widths))
    return _COMPILED[key]


def kernel(z, company_id, W1, b1, W2, b2):
    import ml_dtypes
    from concourse.bass_utils import run_bass_kernel_spmd

    z = np.asarray(z, dtype=np.float32)
    cid = np.asarray(company_id).astype(np.int64).ravel()
    W1 = np.asarray(W1, dtype=np.float32)
    b1 = np.asarray(b1, dtype=np.float32)
    W2 = np.asarray(W2, dtype=np.float32)
    b2 = np.asarray(b2, dtype=np.float32)
    O = W2.shape[2]

    idx_by_company = [np.nonzero(cid == gc)[0] for gc in range(C)]

    # Segment any company with >128 tokens (rare) into <=128-token chunks.
    segs = []  # (gc, tok_start, seg_len)
    for gc in range(C):
        n = len(idx_by_company[gc])
        st = 0
        while st < n or (st == 0 and n == 0):
            ln = min(128, n - st)
            segs.append((gc, st, ln))
            st += max(ln, 1)
            if n == 0:
                break
    while len(segs) % NCORES != 0:
        segs.append((0, 0, 0))

    # Sort descending; slot k gets segs[8k:8k+8] (one per core); shared width.
    segs.sort(key=lambda t: -t[2])
    NSLOT = len(segs) // NCORES
    widths = []
    for k in range(NSLOT):
        mx = max(t[2] for t in segs[k * NCORES:(k + 1) * NCORES])
        widths.append(max(2, ((mx + 1) // 2) * 2))
    cum = np.concatenate([[0], np.cumsum(widths)])
    NTOT = int(cum[-1])
    B1VW = NSLOT * 2 * KC

    nc = _get_compiled(widths)

    in_maps = []
    core_slots = []
    for core in range(NCORES):
        slots = [segs[k * NCORES + core] for k in range(NSLOT)]
        core_slots.append(slots)

        zt = np.zeros((128, KC * NTOT), dtype=np.float16)
        w1 = np.zeros((NSLOT, 128, 2 * KC * (H // 2)),
                      dtype=ml_dtypes.float8_e3m4)
        b1v = np.zeros((128, NSLOT, 2, KC), dtype=np.float16)
        w2h = np.zeros((128, NSLOT * HC), dtype=np.float16)

        for s, (gc, st, ln) in enumerate(slots):
            W = widths[s]
            if ln > 0:
                ix = idx_by_company[gc][st:st + ln]
                zb = np.zeros((KC, 128, W), dtype=np.float16)
                zb[:, :, :ln] = (
                    z[ix].reshape(ln, KC, 128).transpose(1, 2, 0)
                )
                zt[:, KC * cum[s]:KC * (cum[s] + W)] = (
                    zb.transpose(1, 0, 2).reshape(128, KC * W)
                )
            w1[s] = (
                (W1[gc] * SCALE)
                .reshape(KC, 128, 2, H // 2)
                .transpose(1, 2, 0, 3)
                .reshape(128, 2 * KC * (H // 2))
                .astype(ml_dtypes.float8_e3m4)
            )
            # b1v[m, s, g, k] = SCALE*b1[gc, 512g+128k+m]
            b1v[:, s] = (
                (b1[gc] * SCALE).reshape(2, KC, 128).transpose(2, 0, 1)
            ).astype(np.float16)
            w2h[:, HC * s:HC * (s + 1)] = (
                W2[gc, :, 0].reshape(HC, 128).T.astype(np.float16)
            )

        in_maps.append({
            "zt": np.ascontiguousarray(zt),
            "w1": np.ascontiguousarray(w1),
            "b1v": np.ascontiguousarray(b1v.reshape(128, B1VW)),
            "w2": np.ascontiguousarray(w2h),
        })

    res = run_bass_kernel_spmd(nc, in_maps, list(range(NCORES)))

    out = np.zeros((B, O), dtype=np.float32)
    for core in range(NCORES):
        core_out = res.results[core]["out"].reshape(-1)
        for s, (gc, st, ln) in enumerate(core_slots[core]):
            if ln == 0:
                continue
            ix = idx_by_company[gc][st:st + ln]
            out[ix, 0] = core_out[cum[s]:cum[s] + ln] + b2[gc, 0]
    return out
